# revision 18
# baseline (speedup 1.0000x reference)
"""Trainium2 Bass kernel for nn_BranchedNetwork (moe_routing).

Computation (reference):
    meas_embs = measurements @ W_meas + b_meas           [B, 512]
    embs      = concat([img_embs, meas_embs], axis=1)    [B, 1024]
    h_e       = relu(embs @ W1[e] + b1[e])               per expert e
    out_e     = h_e @ W2[e] + b2[e]
    p[i]      = out[command[i], i, 0]
    angle     = sigmoid(p) * 50 ; speed = clip(p, -1, 1)

Strategy (v2 — expert-sharded):
  * Routing on the host: samples grouped by command id; expert e's
    samples are split between cores 2e and 2e+1, so each core runs
    exactly ONE expert and loads only that expert's weights (512 KB
    instead of 2 MB — 4x less weight DMA than data-parallel).
  * Only column 0 of W2 is needed; the second layer reduces to
    p = sum_j w2_j * relu(h_j), computed per 128-row tile by
    ACT (relu, psum->sbuf bf16) + DVE tensor_tensor_reduce
    (multiply by a broadcast signed-w2 row, free-axis accumulate,
    initial value = b2).  All expert-specific numbers travel as
    DATA, so the single SPMD program is uniform across cores.
  * Measurement path folded: h = img @ A[e] + measAug @ WfAug[e]
    where measAug = [meas; 1] (9 rows) and WfAug = [W_meas@W1h; b_eff].
    The K=9 matmuls of 4 consecutive tiles run concurrently in
    separate PE row-groups via tile_position.
  * bf16 operands (fp32 PSUM accumulation), host-pre-tiled layouts so
    every DMA is a dense 2D copy, DMAs placed explicitly on the three
    queue rings (SP/ACT hwdge + Pool swdge) ordered so tile 0's data
    lands first, a short PE warmup bridges the initial DMA window so
    the HAM clock gate is released early, ACT tables preloaded off the
    critical path, and the framework's end-of-kernel tail stripped.
"""

import os
import sys
import types

import numpy as np

if "/opt/trn_rl_repo" not in sys.path and not any(
    p.endswith("trn_rl_repo") for p in sys.path
):
    sys.path.insert(0, "/opt/trn_rl_repo")

B = 16384
EMB = 512
NUM_COMMANDS = 4
NUM_MEAS = 8
NCORES = 8
P = 128

N_WARM = int(os.environ.get("KERNEL_WARM", "5"))

_CACHE = {}


def _install_ntff_shim():
    """Recreate antenv.axon_hooks so trace=True works if requested."""
    if "antenv.axon_hooks" in sys.modules:
        return
    try:
        import antenv

        mod = types.ModuleType("antenv.axon_hooks")
        mod._hook = None
        mod.set_axon_ntff_profile_hook = lambda h: setattr(mod, "_hook", h)
        mod.get_axon_ntff_profile_hook = lambda: mod._hook
        sys.modules["antenv.axon_hooks"] = mod
        antenv.axon_hooks = mod
        from trn_agent_boot.trn_boot import _ntff_profile_via_ctypes

        mod.set_axon_ntff_profile_hook(
            _ntff_profile_via_ctypes("/opt/axon/libaxon_pjrt.so")
        )
    except Exception:
        pass


def _split_excess_waits(nc, max_waits=1):
    """The walrus in this container rejects instructions with more than
    one embedded sync-wait command.  Waits execute in order on the
    issuing engine, so hoisting the excess onto preceding NOPs on the
    same engine is semantically identical."""
    from concourse import mybir

    n_split = 0
    for f in nc.m.functions:
        for bb in f.blocks:
            insts = list(bb.instructions)
            new_insts = []
            changed = False
            for inst in insts:
                si = inst.sync_info
                if si is not None and si.on_wait and len(si.on_wait) > max_waits:
                    waits = list(si.on_wait)
                    extra, keep = waits[:-max_waits], waits[-max_waits:]
                    while extra:
                        chunk, extra = extra[:max_waits], extra[max_waits:]
                        n_split += 1
                        nop = mybir.InstNoOp(
                            name=f"waitsplit_{n_split}_{inst.name}",
                            engine=inst.engine,
                            ins=[],
                            outs=[],
                            sync_info=mybir.SyncInfo(on_wait=chunk, on_update=[]),
                        )
                        new_insts.append(nop)
                    si.on_wait = keep
                    changed = True
                new_insts.append(inst)
            if changed:
                bb.instructions.clear()
                for i in new_insts:
                    bb.instructions.append(i)
    return n_split


def _strip_tail(nc):
    """Remove the end-of-kernel barrier/sem-reset tail.

    The runtime clears semaphores in its own exec preamble, and every
    engine's results flow into the output DMA via data-dependency
    semaphores, so the only thing that must remain is the sync-engine
    DRAIN that flushes the output DMA queue."""
    from concourse import mybir

    f = nc.m.functions[0]
    bb = f.blocks[-1]
    insts = list(bb.instructions)
    idx = None
    for i, inst in enumerate(insts):
        if isinstance(inst, mybir.InstDrain) and inst.engine == mybir.EngineType.SP:
            idx = i
            break
    if idx is None:
        return 0
    kept = insts[: idx + 1]
    drain = kept[-1]
    if drain.sync_info is not None:
        drain.sync_info.on_wait = []
    removed = len(insts) - len(kept)
    bb.instructions.clear()
    for i in kept:
        bb.instructions.append(i)
    return removed


def _np_bf16():
    import ml_dtypes

    return ml_dtypes.bfloat16


def _route(command):
    """Group sample indices by expert; expert e -> cores 2e, 2e+1.

    Returns R (uniform rows per core, multiple of 128) and
    I [NCORES, R] global row-index arrays (padded with repeats)."""
    halves = []
    for e in range(NUM_COMMANDS):
        idx = np.nonzero(command == e)[0].astype(np.int64)
        n = len(idx)
        h = (n + 1) // 2
        halves.append(idx[:h])
        halves.append(idx[h:])
    R = max(P, int(np.ceil(max(len(h) for h in halves) / P)) * P)
    I = []
    for h in halves:
        if len(h) == 0:
            h = np.zeros(1, np.int64)
        pad = R - len(h)
        I.append(np.concatenate([h, np.full(pad, h[-1], np.int64)]))
    return R, np.stack(I)


def _build_program(R):
    from contextlib import ExitStack

    import concourse.bass as bass
    import concourse.tile as tile
    from concourse import mybir

    f32 = mybir.dt.float32
    MMD = mybir.dt.bfloat16
    T = R // P

    nc = bass.Bass()
    # all arrays are PRE-TILED on the host so every DMA is a dense
    # [partition, contiguous-bytes] copy (cheap descriptor generation)
    img_d = nc.declare_dram_parameter("img_pre", [P, 4 * R], MMD, isOutput=False)
    mw_d = nc.declare_dram_parameter(
        "meaWf", [NUM_MEAS + 1, R + EMB], MMD, isOutput=False
    )
    A_d = nc.declare_dram_parameter("A_pre", [P, 4 * EMB], MMD, isOutput=False)
    w2b_d = nc.declare_dram_parameter("w2b", [P, EMB], MMD, isOutput=False)
    b2c_d = nc.declare_dram_parameter("b2c", [P, 1], f32, isOutput=False)
    outp_d = nc.declare_dram_parameter("outp", [P, 2, T], f32, isOutput=True)

    # img chunk boundaries in tiles (chunks of one 4-tile group)
    bounds = [0]
    while bounds[-1] < T:
        bounds.append(min(T, bounds[-1] + 4))
    nchunk = len(bounds) - 1

    with tile.TileContext(nc) as tc:
        with ExitStack() as ctx:
            const_pool = ctx.enter_context(tc.tile_pool(name="const", bufs=1))
            junk_pool = ctx.enter_context(tc.tile_pool(name="junk", bufs=3))
            junk2_pool = ctx.enter_context(tc.tile_pool(name="junk2", bufs=3))
            out_pool = ctx.enter_context(tc.tile_pool(name="out", bufs=1))
            ps_pool = ctx.enter_context(tc.tile_pool(name="ps", bufs=7, space="PSUM"))

            mw_sb = const_pool.tile([P, R + EMB], MMD, tag="meas", name="mw_sb")
            A_sb = const_pool.tile([P, 4 * EMB], MMD, tag="A", name="A_sb")
            w2b_sb = const_pool.tile([P, EMB], MMD, tag="w2b", name="w2b_sb")
            b2c_sb = const_pool.tile([P, 1], f32, tag="b2c", name="b2c_sb")
            img_sb = const_pool.tile([P, 4 * R], MMD, tag="img", name="img_sb")
            pcol = out_pool.tile([P, T], f32, tag="pcol", name="pcol")

            # --- DMA issue.  Queue facts (measured): the SP HWDGE ring
            # starts draining ~1.5us after its first issue; the ACT HWDGE
            # ring lags a further ~1.6us (shared descriptor generation);
            # the Pool SWDGE ring is independent.  Issue slots are ~0.7us
            # each, so the critical prefix must be few, large transfers:
            #   sync (fast):   A, then img bulk
            #   gpsimd (indep): img group 0/1, later img group 3
            #   scalar (lags): meas+Wf replicas, b2c, w2b (all needed
            #                  only after the first img-MM wave)
            warm_a = const_pool.tile([P, EMB], MMD, tag="warm_a", name="warm_a")
            with tc.high_priority():
                nc.gpsimd.memset(warm_a[:], 0.0)
            # A split so the first tile's first matmul is gated by a small
            # transfer whose completion semaphore fires early
            nc.sync.dma_start(A_sb[:, :EMB], A_d[:, :EMB])
            nc.sync.dma_start(A_sb[:, EMB:], A_d[:, EMB:])
            # img group 0 split the same way: tile 0 first
            nc.gpsimd.dma_start(img_sb[:, : 4 * P], img_d[:, : 4 * P])
            nc.gpsimd.dma_start(
                img_sb[:, 4 * P : bounds[1] * 4 * P],
                img_d[:, 4 * P : bounds[1] * 4 * P],
            )
            img_q = [None, nc.gpsimd, nc.sync, nc.gpsimd, nc.sync, nc.sync]
            for ci in range(1, nchunk):
                c0, c1 = bounds[ci] * 4 * P, bounds[ci + 1] * 4 * P
                img_q[ci].dma_start(img_sb[:, c0:c1], img_d[:, c0:c1])
            for j in range(4):
                nc.scalar.dma_start(
                    mw_sb[32 * j : 32 * j + NUM_MEAS + 1, :], mw_d[:]
                )
            nc.scalar.dma_start(b2c_sb[:], b2c_d[:])
            nc.scalar.dma_start(w2b_sb[:], w2b_d[:])

            # --- PE warmup: release the HAM clock gate during the DMA
            # window so the real matmul stream starts at 2.4 GHz
            ps_w = ps_pool.tile([P, EMB], f32, tag="h", name="ps_warm")
            for w in range(N_WARM):
                nc.tensor.matmul(
                    ps_w[:],
                    lhsT=warm_a[:, :P],
                    rhs=warm_a[:],
                    start=(w == 0),
                    stop=(w == N_WARM - 1),
                )
            jw = junk_pool.tile([P, EMB], MMD, tag="junk")
            nc.scalar.activation(
                jw[:, :1], ps_w[:, :1], mybir.ActivationFunctionType.Relu
            )
            nc.scalar.activation(
                jw[:, 1:2], ps_w[:, 1:2], mybir.ActivationFunctionType.Sigmoid
            )

            # --- main loop: groups of 4 tiles share one packed meas pass;
            # each group's angle/speed columns are produced and DMA'd out
            # as soon as its reductions land (short final tail).
            sig = out_pool.tile([P, T], f32, tag="sig", name="sig")
            q = out_pool.tile([P, T], f32, tag="q", name="q")
            outs = out_pool.tile([P, 2, T], f32, tag="outs", name="outs")
            for g0 in range(0, T, 4):
                group = list(range(g0, min(g0 + 4, T)))
                g1 = group[-1] + 1
                ps_of = {}
                # img contraction first (start=True on the first chunk) so
                # the group isn't gated on the meas replicas; the packed
                # K=9 meas matmuls close each accumulation (stop=True) in
                # concurrent PE row-groups at the end of the group.
                for r in group:
                    psr = ps_pool.tile([P, EMB], f32, tag="h", name=f"ps_{r}")
                    ps_of[r] = psr
                    for ko in range(4):
                        nc.tensor.matmul(
                            psr[:],
                            lhsT=img_sb[:, (r * 4 + ko) * P : (r * 4 + ko + 1) * P],
                            rhs=A_sb[:, ko * EMB : (ko + 1) * EMB],
                            start=(ko == 0),
                            stop=False,
                        )
                for j, r in enumerate(group):
                    nc.tensor.matmul(
                        ps_of[r][:],
                        lhsT=mw_sb[
                            32 * j : 32 * j + NUM_MEAS + 1, r * P : (r + 1) * P
                        ],
                        rhs=mw_sb[32 * j : 32 * j + NUM_MEAS + 1, R : R + EMB],
                        start=False,
                        stop=True,
                        tile_position=(32 * j, 0),
                    )
                for r in group:
                    junk = junk_pool.tile([P, EMB], MMD, tag="junk")
                    nc.scalar.activation(
                        junk[:], ps_of[r][:], mybir.ActivationFunctionType.Relu
                    )
                    junk2 = junk2_pool.tile([P, EMB], MMD, tag="junk2")
                    nc.vector.scalar_tensor_tensor(
                        junk2[:],
                        junk[:],
                        0.0,
                        w2b_sb[:],
                        mybir.AluOpType.add,
                        mybir.AluOpType.mult,
                        accum_out=pcol[:, r : r + 1],
                    )
                # group epilogue (b2 folded as per-partition activation bias)
                gs = slice(g0, g1)
                nc.scalar.activation(
                    sig[:, gs],
                    pcol[:, gs],
                    mybir.ActivationFunctionType.Sigmoid,
                    bias=b2c_sb[:, 0:1],
                )
                nc.vector.tensor_scalar_add(q[:, gs], pcol[:, gs], b2c_sb[:, 0:1])
                nc.vector.tensor_scalar_mul(outs[:, 0, gs], sig[:, gs], 50.0)
                nc.vector.tensor_scalar(
                    outs[:, 1, gs],
                    q[:, gs],
                    1.0,
                    -1.0,
                    mybir.AluOpType.min,
                    mybir.AluOpType.max,
                )
            # bulk of the output early; only the last group's columns ride
            # the final (tiny) transfer
            Tcut = (T - 1) // 4 * 4
            if Tcut > 0:
                nc.sync.dma_start(outp_d[:, :, :Tcut], outs[:, :, :Tcut])
            nc.sync.dma_start(outp_d[:, :, Tcut:], outs[:, :, Tcut:])

    _strip_tail(nc)
    _split_excess_waits(nc)
    return nc


def _prepare(inputs):
    img_embs = np.asarray(inputs["img_embs"], np.float32)
    measurements = np.asarray(inputs["measurements"], np.float32)
    command = np.asarray(inputs["command"])
    W_meas = np.asarray(inputs["W_meas"], np.float32)
    b_meas = np.asarray(inputs["b_meas"], np.float32)
    W1 = np.asarray(inputs["W1"], np.float32)
    b1 = np.asarray(inputs["b1"], np.float32)
    W2 = np.asarray(inputs["W2"], np.float32)
    b2 = np.asarray(inputs["b2"], np.float32)

    bf16 = _np_bf16()
    R, I = _route(command)
    T = R // P

    # fold measurement path (float64 for the host-side precompute)
    W1h = W1[:, EMB:, :].astype(np.float64)
    Wf = np.einsum("md,edh->emh", W_meas.astype(np.float64), W1h)
    b_eff = np.einsum("d,edh->eh", b_meas.astype(np.float64), W1h) + b1
    WfAug = np.concatenate([Wf, b_eff[:, None, :]], axis=1).astype(bf16)  # [E,9,512]

    # per-expert shared tensors
    A_pre = {}
    w2b = {}
    b2c = {}
    for e in range(NUM_COMMANDS):
        A_pre[e] = np.ascontiguousarray(
            W1[e, :EMB, :].reshape(4, P, EMB).transpose(1, 0, 2).reshape(P, 4 * EMB)
        ).astype(bf16)
        w2b[e] = np.ascontiguousarray(
            np.broadcast_to(W2[e, :, 0][None, :], (P, EMB))
        ).astype(bf16)
        b2c[e] = np.full((P, 1), b2[e, 0], np.float32)

    imgT = img_embs.T.astype(bf16)  # [512, B]
    measT = measurements.T  # [8, B]
    ones_row = np.ones((1, R), np.float32).astype(bf16)
    in_maps = []
    for k in range(NCORES):
        e = k // 2
        Ik = I[k]
        img_pre = np.ascontiguousarray(
            imgT[:, Ik].reshape(4, P, T, P).transpose(1, 2, 0, 3).reshape(P, 4 * R)
        )
        measAug_k = np.concatenate([measT[:, Ik].astype(bf16), ones_row], axis=0)
        meaWf_k = np.ascontiguousarray(
            np.concatenate([measAug_k, WfAug[e]], axis=1)
        )  # [9, R + 512]
        in_maps.append(
            {
                "img_pre": img_pre,
                "meaWf": meaWf_k,
                "A_pre": A_pre[e],
                "w2b": w2b[e],
                "b2c": b2c[e],
            }
        )
    return in_maps, I, R


def _run(inputs, trace=False):
    """Returns ((angle, speed), BassKernelResults)."""
    _install_ntff_shim()
    from concourse.bass_utils import run_bass_kernel_spmd

    in_maps, I, R = _prepare(inputs)
    if R not in _CACHE:
        _CACHE[R] = _build_program(R)
    nc = _CACHE[R]

    res = run_bass_kernel_spmd(
        nc, in_maps, core_ids=list(range(NCORES)), trace=trace
    )

    nb = int(np.asarray(inputs["command"]).shape[0])
    angle = np.zeros(nb, np.float32)
    speed = np.zeros(nb, np.float32)
    for k in range(NCORES):
        outp = res.results[k]["outp"]  # [128, 2, T]
        Ik = I[k]
        angle[Ik] = outp[:, 0, :].T.reshape(R)
        speed[Ik] = outp[:, 1, :].T.reshape(R)
    return (angle, speed), res


def kernel(**inputs):
    out, _ = _run(inputs)
    return out


# revision 21
# speedup vs baseline: 3.2343x; 3.2343x over previous
"""Trainium2 Bass kernel for nn_BranchedNetwork (moe_routing).

Computation (reference):
    meas_embs = measurements @ W_meas + b_meas           [B, 512]
    embs      = concat([img_embs, meas_embs], axis=1)    [B, 1024]
    h_e       = relu(embs @ W1[e] + b1[e])               per expert e
    out_e     = h_e @ W2[e] + b2[e]
    p[i]      = out[command[i], i, 0]
    angle     = sigmoid(p) * 50 ; speed = clip(p, -1, 1)

Strategy (v2 — expert-sharded):
  * Routing on the host: samples grouped by command id; expert e's
    samples are split between cores 2e and 2e+1, so each core runs
    exactly ONE expert and loads only that expert's weights (512 KB
    instead of 2 MB — 4x less weight DMA than data-parallel).
  * Only column 0 of W2 is needed; the second layer reduces to
    p = sum_j w2_j * relu(h_j), computed per 128-row tile by
    ACT (relu, psum->sbuf bf16) + DVE tensor_tensor_reduce
    (multiply by a broadcast signed-w2 row, free-axis accumulate,
    initial value = b2).  All expert-specific numbers travel as
    DATA, so the single SPMD program is uniform across cores.
  * Measurement path folded: h = img @ A[e] + measAug @ WfAug[e]
    where measAug = [meas; 1] (9 rows) and WfAug = [W_meas@W1h; b_eff].
    The K=9 matmuls of 4 consecutive tiles run concurrently in
    separate PE row-groups via tile_position.
  * bf16 operands (fp32 PSUM accumulation), host-pre-tiled layouts so
    every DMA is a dense 2D copy, DMAs placed explicitly on the three
    queue rings (SP/ACT hwdge + Pool swdge) ordered so tile 0's data
    lands first, a short PE warmup bridges the initial DMA window so
    the HAM clock gate is released early, ACT tables preloaded off the
    critical path, and the framework's end-of-kernel tail stripped.
"""

import os
import sys
import types

import numpy as np

if "/opt/trn_rl_repo" not in sys.path and not any(
    p.endswith("trn_rl_repo") for p in sys.path
):
    sys.path.insert(0, "/opt/trn_rl_repo")

B = 16384
EMB = 512
NUM_COMMANDS = 4
NUM_MEAS = 8
NCORES = 8
P = 128

N_WARM = int(os.environ.get("KERNEL_WARM", "8"))

_CACHE = {}


def _install_ntff_shim():
    """Recreate antenv.axon_hooks so trace=True works if requested."""
    if "antenv.axon_hooks" in sys.modules:
        return
    try:
        import antenv

        mod = types.ModuleType("antenv.axon_hooks")
        mod._hook = None
        mod.set_axon_ntff_profile_hook = lambda h: setattr(mod, "_hook", h)
        mod.get_axon_ntff_profile_hook = lambda: mod._hook
        sys.modules["antenv.axon_hooks"] = mod
        antenv.axon_hooks = mod
        from trn_agent_boot.trn_boot import _ntff_profile_via_ctypes

        mod.set_axon_ntff_profile_hook(
            _ntff_profile_via_ctypes("/opt/axon/libaxon_pjrt.so")
        )
    except Exception:
        pass


def _split_excess_waits(nc, max_waits=1):
    """The walrus in this container rejects instructions with more than
    one embedded sync-wait command.  Waits execute in order on the
    issuing engine, so hoisting the excess onto preceding NOPs on the
    same engine is semantically identical."""
    from concourse import mybir

    n_split = 0
    for f in nc.m.functions:
        for bb in f.blocks:
            insts = list(bb.instructions)
            new_insts = []
            changed = False
            for inst in insts:
                si = inst.sync_info
                if si is not None and si.on_wait and len(si.on_wait) > max_waits:
                    waits = list(si.on_wait)
                    extra, keep = waits[:-max_waits], waits[-max_waits:]
                    while extra:
                        chunk, extra = extra[:max_waits], extra[max_waits:]
                        n_split += 1
                        nop = mybir.InstNoOp(
                            name=f"waitsplit_{n_split}_{inst.name}",
                            engine=inst.engine,
                            ins=[],
                            outs=[],
                            sync_info=mybir.SyncInfo(on_wait=chunk, on_update=[]),
                        )
                        new_insts.append(nop)
                    si.on_wait = keep
                    changed = True
                new_insts.append(inst)
            if changed:
                bb.instructions.clear()
                for i in new_insts:
                    bb.instructions.append(i)
    return n_split


def _strip_tail(nc):
    """Remove the end-of-kernel barrier/sem-reset tail.

    The runtime clears semaphores in its own exec preamble, and every
    engine's results flow into the output DMA via data-dependency
    semaphores, so the only thing that must remain is the sync-engine
    DRAIN that flushes the output DMA queue."""
    from concourse import mybir

    f = nc.m.functions[0]
    bb = f.blocks[-1]
    insts = list(bb.instructions)
    idx = None
    for i, inst in enumerate(insts):
        if isinstance(inst, mybir.InstDrain) and inst.engine == mybir.EngineType.SP:
            idx = i
            break
    if idx is None:
        return 0
    kept = insts[: idx + 1]
    drain = kept[-1]
    if drain.sync_info is not None:
        drain.sync_info.on_wait = []
    removed = len(insts) - len(kept)
    bb.instructions.clear()
    for i in kept:
        bb.instructions.append(i)
    return removed


def _np_bf16():
    import ml_dtypes

    return ml_dtypes.bfloat16


def _route(command):
    """Group sample indices by expert; expert e -> cores 2e, 2e+1.

    Returns R (uniform rows per core, multiple of 128) and
    I [NCORES, R] global row-index arrays (padded with repeats)."""
    halves = []
    for e in range(NUM_COMMANDS):
        idx = np.nonzero(command == e)[0].astype(np.int64)
        n = len(idx)
        h = (n + 1) // 2
        halves.append(idx[:h])
        halves.append(idx[h:])
    R = max(P, int(np.ceil(max(len(h) for h in halves) / P)) * P)
    I = []
    for h in halves:
        if len(h) == 0:
            h = np.zeros(1, np.int64)
        pad = R - len(h)
        I.append(np.concatenate([h, np.full(pad, h[-1], np.int64)]))
    return R, np.stack(I)


def _build_program(R):
    from contextlib import ExitStack

    import concourse.bass as bass
    import concourse.tile as tile
    from concourse import mybir

    f32 = mybir.dt.float32
    MMD = mybir.dt.bfloat16
    T = R // P

    nc = bass.Bass()
    # all arrays are PRE-TILED on the host so every DMA is a dense
    # [partition, contiguous-bytes] copy (cheap descriptor generation)
    img_d = nc.declare_dram_parameter("img_pre", [P, 4 * R], MMD, isOutput=False)
    mw_d = nc.declare_dram_parameter(
        "meaWf", [NUM_MEAS + 1, R + EMB], MMD, isOutput=False
    )
    A_d = nc.declare_dram_parameter("A_pre", [P, 4 * EMB], MMD, isOutput=False)
    w2b_d = nc.declare_dram_parameter("w2b", [P, EMB], MMD, isOutput=False)
    b2c_d = nc.declare_dram_parameter("b2c", [P, 1], f32, isOutput=False)
    outp_d = nc.declare_dram_parameter("outp", [P, 2, T], f32, isOutput=True)

    # img chunk boundaries in tiles (chunks of one 4-tile group)
    bounds = [0]
    while bounds[-1] < T:
        bounds.append(min(T, bounds[-1] + 4))
    nchunk = len(bounds) - 1

    with tile.TileContext(nc) as tc:
        with ExitStack() as ctx:
            const_pool = ctx.enter_context(tc.tile_pool(name="const", bufs=1))
            junk_pool = ctx.enter_context(tc.tile_pool(name="junk", bufs=3))
            junk2_pool = ctx.enter_context(tc.tile_pool(name="junk2", bufs=3))
            out_pool = ctx.enter_context(tc.tile_pool(name="out", bufs=1))
            ps_pool = ctx.enter_context(tc.tile_pool(name="ps", bufs=7, space="PSUM"))

            mw_sb = const_pool.tile([P, R + EMB], MMD, tag="meas", name="mw_sb")
            A_sb = const_pool.tile([P, 4 * EMB], MMD, tag="A", name="A_sb")
            w2b_sb = const_pool.tile([P, EMB], MMD, tag="w2b", name="w2b_sb")
            b2c_sb = const_pool.tile([P, 1], f32, tag="b2c", name="b2c_sb")
            img_sb = const_pool.tile([P, 4 * R], MMD, tag="img", name="img_sb")
            pcol = out_pool.tile([P, T], f32, tag="pcol", name="pcol")

            # --- DMA issue.  Queue facts (measured): the SP HWDGE ring
            # starts draining ~1.5us after its first issue; the ACT HWDGE
            # ring lags a further ~1.6us (shared descriptor generation);
            # the Pool SWDGE ring is independent.  Issue slots are ~0.7us
            # each, so the critical prefix must be few, large transfers:
            #   sync (fast):   A, then img bulk
            #   gpsimd (indep): img group 0/1, later img group 3
            #   scalar (lags): meas+Wf replicas, b2c, w2b (all needed
            #                  only after the first img-MM wave)
            warm_a = const_pool.tile([P, EMB], MMD, tag="warm_a", name="warm_a")
            with tc.high_priority():
                nc.gpsimd.memset(warm_a[:], 0.0)
            # A split so each K-chunk's gating semaphore fires as early as
            # possible (completion = last-byte receipt, ~2-3us after data)
            nc.sync.dma_start(A_sb[:, :EMB], A_d[:, :EMB])
            nc.sync.dma_start(A_sb[:, EMB : 2 * EMB], A_d[:, EMB : 2 * EMB])
            nc.sync.dma_start(A_sb[:, 2 * EMB :], A_d[:, 2 * EMB :])
            # img group 0 split the same way: tile 0 first
            nc.gpsimd.dma_start(img_sb[:, : 4 * P], img_d[:, : 4 * P])
            nc.gpsimd.dma_start(
                img_sb[:, 4 * P : bounds[1] * 4 * P],
                img_d[:, 4 * P : bounds[1] * 4 * P],
            )
            img_q = [None, nc.gpsimd, nc.sync, nc.gpsimd, nc.sync, nc.sync]
            for ci in range(1, nchunk):
                c0, c1 = bounds[ci] * 4 * P, bounds[ci + 1] * 4 * P
                img_q[ci].dma_start(img_sb[:, c0:c1], img_d[:, c0:c1])
            for j in range(4):
                nc.scalar.dma_start(
                    mw_sb[32 * j : 32 * j + NUM_MEAS + 1, :], mw_d[:]
                )
            nc.scalar.dma_start(b2c_sb[:], b2c_d[:])
            nc.scalar.dma_start(w2b_sb[:], w2b_d[:])

            # --- PE warmup: release the HAM clock gate during the DMA
            # window so the real matmul stream starts at 2.4 GHz
            ps_w = ps_pool.tile([P, EMB], f32, tag="h", name="ps_warm")
            for w in range(N_WARM):
                nc.tensor.matmul(
                    ps_w[:],
                    lhsT=warm_a[:, :P],
                    rhs=warm_a[:],
                    start=(w == 0),
                    stop=(w == N_WARM - 1),
                )
            jw = junk_pool.tile([P, EMB], MMD, tag="junk")
            nc.scalar.activation(
                jw[:, :1], ps_w[:, :1], mybir.ActivationFunctionType.Relu
            )
            nc.scalar.activation(
                jw[:, 1:2], ps_w[:, 1:2], mybir.ActivationFunctionType.Sigmoid
            )

            # --- main loop: groups of 4 tiles share one packed meas pass;
            # each group's angle/speed columns are produced and DMA'd out
            # as soon as its reductions land (short final tail).
            sig = out_pool.tile([P, T], f32, tag="sig", name="sig")
            q = out_pool.tile([P, T], f32, tag="q", name="q")
            outs = out_pool.tile([P, 2, T], f32, tag="outs", name="outs")
            for g0 in range(0, T, 4):
                group = list(range(g0, min(g0 + 4, T)))
                g1 = group[-1] + 1
                ps_of = {}
                # img contraction first (start=True on the first chunk) so
                # the group isn't gated on the meas replicas; the packed
                # K=9 meas matmuls close each accumulation (stop=True) in
                # concurrent PE row-groups at the end of the group.
                for r in group:
                    psr = ps_pool.tile([P, EMB], f32, tag="h", name=f"ps_{r}")
                    ps_of[r] = psr
                    for ko in range(4):
                        nc.tensor.matmul(
                            psr[:],
                            lhsT=img_sb[:, (r * 4 + ko) * P : (r * 4 + ko + 1) * P],
                            rhs=A_sb[:, ko * EMB : (ko + 1) * EMB],
                            start=(ko == 0),
                            stop=False,
                        )
                for j, r in enumerate(group):
                    nc.tensor.matmul(
                        ps_of[r][:],
                        lhsT=mw_sb[
                            32 * j : 32 * j + NUM_MEAS + 1, r * P : (r + 1) * P
                        ],
                        rhs=mw_sb[32 * j : 32 * j + NUM_MEAS + 1, R : R + EMB],
                        start=False,
                        stop=True,
                        tile_position=(32 * j, 0),
                    )
                for r in group:
                    junk = junk_pool.tile([P, EMB], MMD, tag="junk")
                    nc.scalar.activation(
                        junk[:], ps_of[r][:], mybir.ActivationFunctionType.Relu
                    )
                    junk2 = junk2_pool.tile([P, EMB], MMD, tag="junk2")
                    nc.vector.scalar_tensor_tensor(
                        junk2[:],
                        junk[:],
                        0.0,
                        w2b_sb[:],
                        mybir.AluOpType.add,
                        mybir.AluOpType.mult,
                        accum_out=pcol[:, r : r + 1],
                    )
                # group epilogue (b2 folded as per-partition activation bias)
                gs = slice(g0, g1)
                nc.scalar.activation(
                    sig[:, gs],
                    pcol[:, gs],
                    mybir.ActivationFunctionType.Sigmoid,
                    bias=b2c_sb[:, 0:1],
                )
                nc.vector.tensor_scalar_add(q[:, gs], pcol[:, gs], b2c_sb[:, 0:1])
                nc.vector.tensor_scalar_mul(outs[:, 0, gs], sig[:, gs], 50.0)
                nc.vector.tensor_scalar(
                    outs[:, 1, gs],
                    q[:, gs],
                    1.0,
                    -1.0,
                    mybir.AluOpType.min,
                    mybir.AluOpType.max,
                )
            nc.sync.dma_start(outp_d[:], outs[:])

    _strip_tail(nc)
    _split_excess_waits(nc)
    return nc


def _prepare(inputs):
    img_embs = np.asarray(inputs["img_embs"], np.float32)
    measurements = np.asarray(inputs["measurements"], np.float32)
    command = np.asarray(inputs["command"])
    W_meas = np.asarray(inputs["W_meas"], np.float32)
    b_meas = np.asarray(inputs["b_meas"], np.float32)
    W1 = np.asarray(inputs["W1"], np.float32)
    b1 = np.asarray(inputs["b1"], np.float32)
    W2 = np.asarray(inputs["W2"], np.float32)
    b2 = np.asarray(inputs["b2"], np.float32)

    bf16 = _np_bf16()
    R, I = _route(command)
    T = R // P

    # fold measurement path (float64 for the host-side precompute)
    W1h = W1[:, EMB:, :].astype(np.float64)
    Wf = np.einsum("md,edh->emh", W_meas.astype(np.float64), W1h)
    b_eff = np.einsum("d,edh->eh", b_meas.astype(np.float64), W1h) + b1
    WfAug = np.concatenate([Wf, b_eff[:, None, :]], axis=1).astype(bf16)  # [E,9,512]

    # per-expert shared tensors
    A_pre = {}
    w2b = {}
    b2c = {}
    for e in range(NUM_COMMANDS):
        A_pre[e] = np.ascontiguousarray(
            W1[e, :EMB, :].reshape(4, P, EMB).transpose(1, 0, 2).reshape(P, 4 * EMB)
        ).astype(bf16)
        w2b[e] = np.ascontiguousarray(
            np.broadcast_to(W2[e, :, 0][None, :], (P, EMB))
        ).astype(bf16)
        b2c[e] = np.full((P, 1), b2[e, 0], np.float32)

    imgT = img_embs.T.astype(bf16)  # [512, B]
    measT = measurements.T  # [8, B]
    ones_row = np.ones((1, R), np.float32).astype(bf16)
    in_maps = []
    for k in range(NCORES):
        e = k // 2
        Ik = I[k]
        img_pre = np.ascontiguousarray(
            imgT[:, Ik].reshape(4, P, T, P).transpose(1, 2, 0, 3).reshape(P, 4 * R)
        )
        measAug_k = np.concatenate([measT[:, Ik].astype(bf16), ones_row], axis=0)
        meaWf_k = np.ascontiguousarray(
            np.concatenate([measAug_k, WfAug[e]], axis=1)
        )  # [9, R + 512]
        in_maps.append(
            {
                "img_pre": img_pre,
                "meaWf": meaWf_k,
                "A_pre": A_pre[e],
                "w2b": w2b[e],
                "b2c": b2c[e],
            }
        )
    return in_maps, I, R


def _run(inputs, trace=False):
    """Returns ((angle, speed), BassKernelResults)."""
    _install_ntff_shim()
    from concourse.bass_utils import run_bass_kernel_spmd

    in_maps, I, R = _prepare(inputs)
    if R not in _CACHE:
        _CACHE[R] = _build_program(R)
    nc = _CACHE[R]

    res = run_bass_kernel_spmd(
        nc, in_maps, core_ids=list(range(NCORES)), trace=trace
    )

    nb = int(np.asarray(inputs["command"]).shape[0])
    angle = np.zeros(nb, np.float32)
    speed = np.zeros(nb, np.float32)
    for k in range(NCORES):
        outp = res.results[k]["outp"]  # [128, 2, T]
        Ik = I[k]
        angle[Ik] = outp[:, 0, :].T.reshape(R)
        speed[Ik] = outp[:, 1, :].T.reshape(R)
    return (angle, speed), res


def kernel(**inputs):
    out, _ = _run(inputs)
    return out
